# revision 14
# baseline (speedup 1.0000x reference)
"""GQA decode attention (b=32, T=4096, 64 q-heads / 8 kv-heads) on 8 trn2 cores.

Tensor-parallel over heads: core i owns kv-head i (q-heads 8i..8i+7),
wqkv block i, KV-cache slice i, wo input-rows 1024i..1024(i+1); a
ReduceScatter finishes the row-parallel wo (host concatenates the shards).

Host-side layout prep (numerically equivalent, layout only):
  - RoPE is linear in q/k for a fixed position, so it is folded into the
    wqkv weight columns (q also absorbs the 1/sqrt(128) score scale).
  - K slice pre-transposed to [b, d, t] so score matmuls contract d on
    partitions; V / wqkv / wo packed partition-major for contiguous
    per-partition DMA runs.
  - Streamed operands cast to bf16 (fp32 PSUM accumulation throughout).
"""

import math
import sys

import numpy as np

sys.path.insert(0, "/opt/trn_rl_repo")

B = 32          # batch
D = 8192        # model dim
HD = 128        # head dim
H = 8           # q-heads per core
NKV = 8         # kv heads (= cores)
T = 4096        # kv length
NT = T // 128   # t-tiles
KD = D // 128   # k-tiles over model dim
BLK = 1280      # wqkv block per kv head (8*128 q | 128 k | 128 v)
KB = 8          # wqkv k-tiles batched per DMA

STREAM_BF16 = True   # stream K/V/weights as bf16 (fp32 accumulate)

_CACHE: dict = {}


def _build():
    from contextlib import ExitStack

    import concourse.tile as tile
    from concourse import bacc, mybir
    from concourse.masks import make_identity

    f32 = mybir.dt.float32
    dt = mybir.dt.bfloat16 if STREAM_BF16 else f32
    nc = bacc.Bacc("TRN2", target_bir_lowering=False, debug=False, num_devices=8)

    f16 = mybir.dt.float16
    xT = nc.dram_tensor("xT", [128, KD, B], dt, kind="ExternalInput")
    wq = nc.dram_tensor("wq", [128, KD, BLK], dt, kind="ExternalInput")
    # K^T and V packed per batch: [:, b, 0:T] = k^T cols, [:, b, T+t*128+d] = v
    kv = nc.dram_tensor("kv", [128, B, 2 * T], dt, kind="ExternalInput")
    woT = nc.dram_tensor("woT", [128, H, D], dt, kind="ExternalInput")
    out_ext = nc.dram_tensor("out", [B // 8, D], f16, kind="ExternalOutput")

    ExpF = mybir.ActivationFunctionType.Exp

    with tile.TileContext(nc) as tc, ExitStack() as ctx:
        cst = ctx.enter_context(tc.tile_pool(name="const", bufs=1))
        ident = cst.tile([128, 128], dt)
        make_identity(nc, ident[:])
        ident32 = cst.tile([8, 8], f32)
        make_identity(nc, ident32[:])
        ones = cst.tile([128, 1], dt)
        nc.vector.memset(ones[:], 1.0)

        # K/V stream pool sits below the phase-1 pools on the SBUF stack so
        # its first DMAs (no data deps) overlap the wqkv streaming.
        kvp = ctx.enter_context(tc.tile_pool(name="kv", bufs=3))
        qT_sb = cst.tile([128, H, B], dt)       # q^T  [d, h, b]
        knT_sb = cst.tile([128, B], dt)         # k_new^T [d, b]
        vn_sb = cst.tile([B, HD], dt)           # v_new [b, d]
        attT_sb = cst.tile([128, H, B], dt)     # att^T [d, h, b]

        # ---------------- phase 1: fused qkv projection ----------------
        with (
            tc.tile_pool(name="w", bufs=3) as wpool,
            tc.tile_pool(name="xt", bufs=1) as xpool,
            tc.tile_pool(name="qps", bufs=1, space="PSUM") as qps,
            tc.tile_pool(name="m1", bufs=1) as m1,
            tc.tile_pool(name="tps", bufs=1, space="PSUM") as tps,
        ):
            xt = xpool.tile([128, KD, B], dt)
            nc.scalar.dma_start(xt[:], xT[:])
            ps_q1 = qps.tile([B, 512], f32)
            ps_q2 = qps.tile([B, 512], f32)
            ps_kv = qps.tile([B, 256], f32)
            for kk in range(0, KD, KB):
                wt = wpool.tile([128, KB, BLK], dt)
                nc.scalar.dma_start(wt[:], wq[:, kk:kk + KB, :])
                for k in range(KB):
                    lhs = xt[:, kk + k, :]
                    st, sp = kk + k == 0, kk + k == KD - 1
                    nc.tensor.matmul(ps_q1[:], lhs, wt[:, k, 0:512], start=st, stop=sp)
                    nc.tensor.matmul(ps_q2[:], lhs, wt[:, k, 512:1024], start=st, stop=sp)
                    nc.tensor.matmul(ps_kv[:], lhs, wt[:, k, 1024:1280], start=st, stop=sp)

            q_sb = m1.tile([B, 1024], dt)
            nc.vector.tensor_copy(q_sb[:, 0:512], ps_q1[:])
            nc.vector.tensor_copy(q_sb[:, 512:1024], ps_q2[:])
            kv_sb = m1.tile([B, 256], dt)
            nc.vector.tensor_copy(kv_sb[:], ps_kv[:])
            nc.vector.tensor_copy(vn_sb[:], kv_sb[:, 128:256])

            t_ps = tps.tile([128, H, B], dt)
            for h in range(H):
                nc.tensor.transpose(
                    t_ps[:, h, :], q_sb[:, h * 128:(h + 1) * 128], ident[0:B, 0:B]
                )
            nc.vector.tensor_copy(qT_sb[:], t_ps[:])
            t2_ps = tps.tile([128, B], dt)
            nc.tensor.transpose(t2_ps[:], kv_sb[:, 0:128], ident[0:B, 0:B])
            nc.vector.tensor_copy(knT_sb[:], t2_ps[:])

        wop = ctx.enter_context(tc.tile_pool(name="wo", bufs=16))

        # ---------------- phase 2: attention over batches ----------------
        with (
            tc.tile_pool(name="pr", bufs=3) as prp,
            tc.tile_pool(name="scps", bufs=3, space="PSUM") as scp,
            tc.tile_pool(name="ovps", bufs=2, space="PSUM") as ovp,
            tc.tile_pool(name="dnps", bufs=1, space="PSUM") as dnp,
            tc.tile_pool(name="atps", bufs=2, space="PSUM") as atp,
            tc.tile_pool(name="att", bufs=2) as attp,
        ):
            wt_tiles = []
            for b in range(B):
                # paced prefetch of wo weight tiles through the attention phase
                if b >= 2 and len(wt_tiles) < 16:
                  i = len(wt_tiles)
                  half, k = divmod(i, H)
                  wt = wop.tile([128, 4096], dt, name="wt", tag="wt")
                  eng = nc.sync if i % 2 == 0 else nc.scalar
                  eng.dma_start(wt[:], woT[:, k, half * 4096:(half + 1) * 4096])
                  wt_tiles.append(wt)
                kv_t = kvp.tile([128, 2 * T], dt, name="kv_t", tag="kv_t")
                eng = nc.sync if b % 2 == 0 else nc.scalar
                eng.dma_start(kv_t[:], kv[:, b, :])
                # overwrite column start_pos with the new (rope'd) k
                nc.vector.tensor_copy(kv_t[:, T - 1:T], knT_sb[:, b:b + 1])
                # overwrite row start_pos (= v tile NT-1, partition 127) with new v
                nc.gpsimd.dma_start(
                    kv_t[127:128, T + (NT - 1) * HD:T + NT * HD], vn_sb[b:b + 1, :]
                )

                sc = scp.tile([128, NT, H], f32)
                for j in range(NT):
                    nc.tensor.matmul(
                        sc[:, j, :], kv_t[:, j * 128:(j + 1) * 128], qT_sb[:, :, b],
                        start=True, stop=True,
                    )
                pr = prp.tile([128, NT, H], dt)
                nc.scalar.activation(pr[:], sc[:], ExpF)

                # denominator: collapse partitions with a ones-stationary matmul,
                # then fold the 32 tiles with a strided DVE reduce, then a tiny
                # PE transpose to land [8, 1] next to the PV output.
                dn1 = dnp.tile([1, NT * H], f32)
                nc.tensor.matmul(
                    dn1[:], ones[:], pr[:, :, :], start=True, stop=True
                )
                den8 = attp.tile([1, H], f32)
                nc.vector.reduce_sum(
                    den8[:], dn1.rearrange("p (t h) -> p h t", h=H),
                    axis=mybir.AxisListType.X,
                )

                ov = ovp.tile([H, HD + 1], f32)
                for j in range(NT):
                    nc.tensor.matmul(
                        ov[:, 0:HD], pr[:, j, :],
                        kv_t[:, T + j * HD:T + (j + 1) * HD],
                        start=(j == 0), stop=(j == NT - 1),
                    )
                nc.tensor.transpose(ov[:, HD:HD + 1], den8[:], ident32[0:1, 0:1])

                rec = attp.tile([H, 1], f32)
                nc.vector.reciprocal(rec[:], ov[:, HD:HD + 1])
                att_b = attp.tile([H, HD], dt)
                nc.vector.tensor_scalar_mul(att_b[:], ov[:, 0:HD], rec[:])
                at_ps = atp.tile([128, H], dt)
                nc.tensor.transpose(at_ps[:], att_b[:], ident[0:H, 0:H])
                nc.vector.tensor_copy(attT_sb[:, :, b], at_ps[:])

        # ---------------- phase 3: wo row-parallel + ReduceScatter ----------------
        # Column quarters pipeline through 2 PSUM slots (matmul q+1 while DVE
        # copies q). Partials cross the collectives in fp16 (half the wire
        # bytes of fp32, 4x less rounding noise than bf16); the wo halves get
        # separate ReduceScatters so the first overlaps the second's matmuls.
        with (
            tc.tile_pool(name="wops", bufs=2, space="PSUM") as wops,
            tc.tile_pool(name="ob", bufs=2) as obp,
            tc.tile_pool(name="dram", bufs=1, space="DRAM") as dram,
        ):
            cc_in = [
                dram.tile([B, D // 2], f16, name=f"cc_in{h}") for h in range(2)
            ]
            cc_out = [
                dram.tile([B // 8, D // 2], f16, name=f"cc_out{h}")
                for h in range(2)
            ]
            for quarter in range(4):
                half, hq = divmod(quarter, 2)
                psq = wops.tile([B, 2048], f32, name="wops", tag="wops")
                for k in range(H):
                    wt = wt_tiles[half * H + k]
                    for n in range(4):
                        cs = hq * 2048 + n * 512
                        nc.tensor.matmul(
                            psq[:, n * 512:(n + 1) * 512], attT_sb[:, k, :],
                            wt[:, cs:cs + 512],
                            start=(k == 0), stop=(k == H - 1),
                        )
                ob = obp.tile([B, 2048], f16, name="ob", tag="ob")
                nc.vector.tensor_copy(ob[:], psq[:])
                nc.sync.dma_start(
                    cc_in[half][:, hq * 2048:(hq + 1) * 2048], ob[:]
                )
                if hq == 1:
                    nc.gpsimd.collective_compute(
                        "ReduceScatter",
                        mybir.AluOpType.add,
                        replica_groups=[list(range(8))],
                        ins=[cc_in[half].opt()],
                        outs=[cc_out[half].opt()],
                    )
                    nc.sync.dma_start(
                        out_ext[:, half * (D // 2):(half + 1) * (D // 2)],
                        cc_out[half][:],
                    )

    nc.compile()
    return nc


def _prep_inputs(x, cache_k, cache_v, wqkv_w, wo_w, freqs_cos, freqs_sin):
    if STREAM_BF16:
        import ml_dtypes

        sdt = ml_dtypes.bfloat16
    else:
        sdt = np.float32
    cos = np.asarray(freqs_cos, np.float32).reshape(-1)[:64]
    sin = np.asarray(freqs_sin, np.float32).reshape(-1)[:64]
    x = np.asarray(x, np.float32).reshape(B, D)
    # x^T packed tile-major: xT[p, k, b] = x[b, 128k+p]
    xT = np.ascontiguousarray(x.reshape(B, KD, 128).transpose(2, 1, 0)).astype(sdt)

    wqkv_w = np.asarray(wqkv_w, np.float32)
    scale = 1.0 / math.sqrt(HD)
    in_maps = []
    for c in range(8):
        W = wqkv_w[:, c * BLK:(c + 1) * BLK].copy()
        q = W[:, :1024].reshape(D, H, 64, 2)
        q0 = q[..., 0].copy()
        q1 = q[..., 1].copy()
        q[..., 0] = (q0 * cos - q1 * sin) * scale
        q[..., 1] = (q0 * sin + q1 * cos) * scale
        k = W[:, 1024:1152].reshape(D, 64, 2)
        k0 = k[..., 0].copy()
        k1 = k[..., 1].copy()
        k[..., 0] = k0 * cos - k1 * sin
        k[..., 1] = k0 * sin + k1 * cos
        # partition-major: wq_pm[p, kt, :] = W[kt*128+p, :]
        W_pm = np.ascontiguousarray(
            W.reshape(KD, 128, BLK).transpose(1, 0, 2)
        ).astype(sdt)

        kTc = np.asarray(cache_k[:, :, c, :], np.float32).transpose(2, 0, 1)
        # [128, B, T] with partition = d
        vc = (
            np.asarray(cache_v[:, :, c, :], np.float32)
            .reshape(B, NT, 128, HD)
            .transpose(2, 0, 1, 3)
            .reshape(128, B, T)
        )  # [128, B, NT*HD] with partition = t % 128
        kv = np.ascontiguousarray(
            np.concatenate([kTc, vc], axis=2)
        ).astype(sdt)  # [128, B, 2T]: k^T cols then v rows
        woTc = np.asarray(wo_w[:, c * 1024:(c + 1) * 1024], np.float32).T  # [1024, D]
        woT_pm = np.ascontiguousarray(
            woTc.reshape(H, 128, D).transpose(1, 0, 2)
        ).astype(sdt)  # [128, H, D]
        in_maps.append({
            "xT": xT, "wq": W_pm, "kv": kv, "woT": woT_pm,
        })
    return in_maps


def kernel(x, cache_k, cache_v, wqkv_w, wo_w, freqs_cos, freqs_sin, mask,
           start_pos, _want_trace=False, **_unused):
    from concourse.bass_utils import run_bass_kernel_spmd

    sp = int(np.asarray(start_pos))
    assert sp == T - 1, f"kernel compiled for start_pos={T - 1}, got {sp}"

    if "nc" not in _CACHE:
        _CACHE["nc"] = _build()
    nc = _CACHE["nc"]

    in_maps = _prep_inputs(x, cache_k, cache_v, wqkv_w, wo_w, freqs_cos, freqs_sin)
    res = run_bass_kernel_spmd(nc, in_maps, list(range(8)), trace=_want_trace)
    # ReduceScatter leaves rank i holding reduced rows 4i..4(i+1): concatenate
    out = np.concatenate(
        [np.asarray(res.results[i]["out"], np.float32) for i in range(8)], axis=0
    )
    out = out.reshape(B, 1, D)
    if _want_trace:
        _CACHE["last_result"] = res
    return out



# revision 24
# speedup vs baseline: 1.0560x; 1.0560x over previous
"""GQA decode attention (b=32, T=4096, 64 q-heads / 8 kv-heads) on 8 trn2 cores.

Tensor-parallel over heads: core i owns kv-head i (q-heads 8i..8i+7),
wqkv block i, KV-cache slice i, wo input-rows 1024i..1024(i+1); a
ReduceScatter finishes the row-parallel wo (host concatenates the shards).

Host-side layout prep (numerically equivalent, layout only):
  - RoPE is linear in q/k for a fixed position, so it is folded into the
    wqkv weight columns (q also absorbs the 1/sqrt(128) score scale).
  - K slice pre-transposed to [b, d, t] so score matmuls contract d on
    partitions; V / wqkv / wo packed partition-major for contiguous
    per-partition DMA runs.
  - Streamed operands cast to bf16 (fp32 PSUM accumulation throughout).
"""

import math
import sys

import numpy as np

sys.path.insert(0, "/opt/trn_rl_repo")

B = 32          # batch
D = 8192        # model dim
HD = 128        # head dim
H = 8           # q-heads per core
NKV = 8         # kv heads (= cores)
T = 4096        # kv length
NT = T // 128   # t-tiles
KD = D // 128   # k-tiles over model dim
BLK = 1280      # wqkv block per kv head (8*128 q | 128 k | 128 v)
KB = 8          # wqkv k-tiles batched per DMA

STREAM_BF16 = True   # stream K/V/weights as bf16 (fp32 accumulate)

_CACHE: dict = {}


def _build():
    from contextlib import ExitStack

    import concourse.tile as tile
    from concourse import bacc, mybir
    from concourse.masks import make_identity

    f32 = mybir.dt.float32
    dt = mybir.dt.bfloat16 if STREAM_BF16 else f32
    nc = bacc.Bacc("TRN2", target_bir_lowering=False, debug=False, num_devices=8)

    xT = nc.dram_tensor("xT", [128, KD, B], dt, kind="ExternalInput")
    wq = nc.dram_tensor("wq", [128, KD, BLK], dt, kind="ExternalInput")
    kT = nc.dram_tensor("kT", [128, B, T], dt, kind="ExternalInput")
    vv = nc.dram_tensor("vv", [128, B, NT, HD], dt, kind="ExternalInput")
    woT = nc.dram_tensor("woT", [128, H, D], dt, kind="ExternalInput")
    out_ext = nc.dram_tensor("out", [B // 8, D], dt, kind="ExternalOutput")

    ExpF = mybir.ActivationFunctionType.Exp

    with tile.TileContext(nc) as tc, ExitStack() as ctx:
        cst = ctx.enter_context(tc.tile_pool(name="const", bufs=1))
        ident = cst.tile([128, 128], dt)
        make_identity(nc, ident[:])
        ident32 = cst.tile([8, 8], f32)
        make_identity(nc, ident32[:])
        ones = cst.tile([128, 1], dt)
        nc.vector.memset(ones[:], 1.0)

        # K/V stream pools sit below the phase-1 pools on the SBUF stack so
        # their first DMAs (no data deps) overlap the wqkv streaming.
        ktp = ctx.enter_context(tc.tile_pool(name="kt", bufs=4))
        vtp = ctx.enter_context(tc.tile_pool(name="vt", bufs=4))
        qT_sb = cst.tile([128, H, B], dt)       # q^T  [d, h, b]
        knT_sb = cst.tile([128, B], dt)         # k_new^T [d, b]
        vn_sb = cst.tile([B, HD], dt)           # v_new [b, d]
        attT_sb = cst.tile([128, H, B], dt)     # att^T [d, h, b]

        # ---------------- phase 1: fused qkv projection ----------------
        with (
            tc.tile_pool(name="w", bufs=3) as wpool,
            tc.tile_pool(name="xt", bufs=1) as xpool,
            tc.tile_pool(name="qps", bufs=1, space="PSUM") as qps,
            tc.tile_pool(name="m1", bufs=1) as m1,
            tc.tile_pool(name="tps", bufs=1, space="PSUM") as tps,
        ):
            xt = xpool.tile([128, KD, B], dt)
            nc.scalar.dma_start(xt[:], xT[:])
            ps_q1 = qps.tile([B, 512], f32)
            ps_q2 = qps.tile([B, 512], f32)
            ps_kv = qps.tile([B, 256], f32)
            for kk in range(0, KD, KB):
                wt = wpool.tile([128, KB, BLK], dt)
                nc.scalar.dma_start(wt[:], wq[:, kk:kk + KB, :])
                for k in range(KB):
                    lhs = xt[:, kk + k, :]
                    st, sp = kk + k == 0, kk + k == KD - 1
                    nc.tensor.matmul(ps_q1[:], lhs, wt[:, k, 0:512], start=st, stop=sp)
                    nc.tensor.matmul(ps_q2[:], lhs, wt[:, k, 512:1024], start=st, stop=sp)
                    nc.tensor.matmul(ps_kv[:], lhs, wt[:, k, 1024:1280], start=st, stop=sp)

            q_sb = m1.tile([B, 1024], dt)
            nc.vector.tensor_copy(q_sb[:, 0:512], ps_q1[:])
            nc.vector.tensor_copy(q_sb[:, 512:1024], ps_q2[:])
            kv_sb = m1.tile([B, 256], dt)
            nc.vector.tensor_copy(kv_sb[:], ps_kv[:])
            nc.vector.tensor_copy(vn_sb[:], kv_sb[:, 128:256])

            t_ps = tps.tile([128, H, B], dt)
            for h in range(H):
                nc.tensor.transpose(
                    t_ps[:, h, :], q_sb[:, h * 128:(h + 1) * 128], ident[0:B, 0:B]
                )
            nc.vector.tensor_copy(qT_sb[:], t_ps[:])
            t2_ps = tps.tile([128, B], dt)
            nc.tensor.transpose(t2_ps[:], kv_sb[:, 0:128], ident[0:B, 0:B])
            nc.vector.tensor_copy(knT_sb[:], t2_ps[:])

        wop = ctx.enter_context(tc.tile_pool(name="wo", bufs=16))

        # ---------------- phase 2: attention over batches ----------------
        with (
            tc.tile_pool(name="pr", bufs=3) as prp,
            tc.tile_pool(name="scps", bufs=3, space="PSUM") as scp,
            tc.tile_pool(name="ovps", bufs=2, space="PSUM") as ovp,
            tc.tile_pool(name="dnps", bufs=1, space="PSUM") as dnp,
            tc.tile_pool(name="atps", bufs=2, space="PSUM") as atp,
            tc.tile_pool(name="att", bufs=2) as attp,
        ):
            wt_tiles = []
            for b in range(B):
                # paced prefetch of wo weight tiles through the attention phase
                if b >= 2 and len(wt_tiles) < 16:
                  i = len(wt_tiles)
                  half, k = divmod(i, H)
                  wt = wop.tile([128, 4096], dt, name="wt", tag="wt")
                  eng = nc.sync if i % 2 == 0 else nc.scalar
                  eng.dma_start(wt[:], woT[:, k, half * 4096:(half + 1) * 4096])
                  wt_tiles.append(wt)
                kt_t = ktp.tile([128, T], dt, name="kt_t", tag="kt_t")
                nc.sync.dma_start(kt_t[:], kT[:, b, :])
                vt_t = vtp.tile([128, NT, HD], dt, name="vt_t", tag="vt_t")
                nc.scalar.dma_start(vt_t[:], vv[:, b, :, :])
                # overwrite column start_pos with the new (rope'd) k
                nc.vector.tensor_copy(kt_t[:, T - 1:T], knT_sb[:, b:b + 1])
                # overwrite row start_pos (= tile NT-1, partition 127) with new v
                nc.gpsimd.dma_start(vt_t[127:128, NT - 1, :], vn_sb[b:b + 1, :])

                sc = scp.tile([128, NT, H], f32)
                for j in range(NT):
                    nc.tensor.matmul(
                        sc[:, j, :], kt_t[:, j * 128:(j + 1) * 128], qT_sb[:, :, b],
                        start=True, stop=True,
                    )
                pr = prp.tile([128, NT, H], dt)
                nc.scalar.activation(pr[:], sc[:], ExpF)

                # denominator: collapse partitions with a ones-stationary matmul,
                # then fold the 32 tiles with a strided DVE reduce, then a tiny
                # PE transpose to land [8, 1] next to the PV output.
                dn1 = dnp.tile([1, NT * H], f32)
                nc.tensor.matmul(
                    dn1[:], ones[:], pr[:, :, :], start=True, stop=True
                )
                den8 = attp.tile([1, H], f32)
                nc.vector.reduce_sum(
                    den8[:], dn1.rearrange("p (t h) -> p h t", h=H),
                    axis=mybir.AxisListType.X,
                )

                ov = ovp.tile([H, HD + 1], f32)
                for j in range(NT):
                    nc.tensor.matmul(
                        ov[:, 0:HD], pr[:, j, :], vt_t[:, j, :],
                        start=(j == 0), stop=(j == NT - 1),
                    )
                nc.tensor.transpose(ov[:, HD:HD + 1], den8[:], ident32[0:1, 0:1])

                rec = attp.tile([H, 1], f32)
                nc.vector.reciprocal(rec[:], ov[:, HD:HD + 1])
                att_b = attp.tile([H, HD], dt)
                nc.vector.tensor_scalar_mul(att_b[:], ov[:, 0:HD], rec[:])
                at_ps = atp.tile([128, H], dt)
                nc.tensor.transpose(at_ps[:], att_b[:], ident[0:H, 0:H])
                nc.vector.tensor_copy(attT_sb[:, :, b], at_ps[:])

        # ---------------- phase 3: wo row-parallel + ReduceScatter ----------------
        # Partials cross the collective in bf16: halves the wire bytes.
        with (
            tc.tile_pool(name="wops", bufs=1, space="PSUM") as wops,
            tc.tile_pool(name="ob", bufs=1) as obp,
            tc.tile_pool(name="dram", bufs=1, space="DRAM") as dram,
        ):
            cc_in = dram.tile([B, D], dt, name="cc_in")
            cc_out = dram.tile([B // 8, D], dt, name="cc_out")
            for half in range(2):
                ps_list = [
                    wops.tile([B, 512], f32, name=f"wops{n}", tag=f"wops{n}")
                    for n in range(8)
                ]
                for k in range(H):
                    wt = wt_tiles[half * H + k]
                    for n in range(8):
                        nc.tensor.matmul(
                            ps_list[n][:], attT_sb[:, k, :], wt[:, n * 512:(n + 1) * 512],
                            start=(k == 0), stop=(k == H - 1),
                        )
                ob = obp.tile([B, 4096], dt, name="ob", tag="ob")
                for n in range(8):
                    nc.vector.tensor_copy(
                        ob[:, n * 512:(n + 1) * 512], ps_list[n][:]
                    )
                hs = slice(half * 4096, (half + 1) * 4096)
                nc.sync.dma_start(cc_in[:, hs], ob[:])
            nc.gpsimd.collective_compute(
                "ReduceScatter",
                mybir.AluOpType.add,
                replica_groups=[list(range(8))],
                ins=[cc_in.opt()],
                outs=[cc_out.opt()],
            )
            nc.sync.dma_start(out_ext[:], cc_out[:])

    nc.compile()
    return nc


def _prep_inputs(x, cache_k, cache_v, wqkv_w, wo_w, freqs_cos, freqs_sin):
    if STREAM_BF16:
        import ml_dtypes

        sdt = ml_dtypes.bfloat16
    else:
        sdt = np.float32
    cos = np.asarray(freqs_cos, np.float32).reshape(-1)[:64]
    sin = np.asarray(freqs_sin, np.float32).reshape(-1)[:64]
    x = np.asarray(x, np.float32).reshape(B, D)
    # x^T packed tile-major: xT[p, k, b] = x[b, 128k+p]
    xT = np.ascontiguousarray(x.reshape(B, KD, 128).transpose(2, 1, 0)).astype(sdt)

    wqkv_w = np.asarray(wqkv_w, np.float32)
    scale = 1.0 / math.sqrt(HD)
    in_maps = []
    for c in range(8):
        W = wqkv_w[:, c * BLK:(c + 1) * BLK].copy()
        q = W[:, :1024].reshape(D, H, 64, 2)
        q0 = q[..., 0].copy()
        q1 = q[..., 1].copy()
        q[..., 0] = (q0 * cos - q1 * sin) * scale
        q[..., 1] = (q0 * sin + q1 * cos) * scale
        k = W[:, 1024:1152].reshape(D, 64, 2)
        k0 = k[..., 0].copy()
        k1 = k[..., 1].copy()
        k[..., 0] = k0 * cos - k1 * sin
        k[..., 1] = k0 * sin + k1 * cos
        # partition-major: wq_pm[p, kt, :] = W[kt*128+p, :]
        W_pm = np.ascontiguousarray(
            W.reshape(KD, 128, BLK).transpose(1, 0, 2)
        ).astype(sdt)

        kTc = np.ascontiguousarray(
            np.asarray(cache_k[:, :, c, :], np.float32).transpose(2, 0, 1)
        ).astype(sdt)  # [128, B, T]
        vc = np.ascontiguousarray(
            np.asarray(cache_v[:, :, c, :], np.float32)
            .reshape(B, NT, 128, HD)
            .transpose(2, 0, 1, 3)
        ).astype(sdt)  # [128, B, NT, HD]
        woTc = np.asarray(wo_w[:, c * 1024:(c + 1) * 1024], np.float32).T  # [1024, D]
        woT_pm = np.ascontiguousarray(
            woTc.reshape(H, 128, D).transpose(1, 0, 2)
        ).astype(sdt)  # [128, H, D]
        in_maps.append({
            "xT": xT, "wq": W_pm, "kT": kTc, "vv": vc, "woT": woT_pm,
        })
    return in_maps


def kernel(x, cache_k, cache_v, wqkv_w, wo_w, freqs_cos, freqs_sin, mask,
           start_pos, _want_trace=False, **_unused):
    from concourse.bass_utils import run_bass_kernel_spmd

    sp = int(np.asarray(start_pos))
    assert sp == T - 1, f"kernel compiled for start_pos={T - 1}, got {sp}"

    if "nc" not in _CACHE:
        _CACHE["nc"] = _build()
    nc = _CACHE["nc"]

    in_maps = _prep_inputs(x, cache_k, cache_v, wqkv_w, wo_w, freqs_cos, freqs_sin)
    res = run_bass_kernel_spmd(nc, in_maps, list(range(8)), trace=_want_trace)
    # ReduceScatter leaves rank i holding reduced rows 4i..4(i+1): concatenate
    out = np.concatenate(
        [np.asarray(res.results[i]["out"], np.float32) for i in range(8)], axis=0
    )
    out = out.reshape(B, 1, D)
    if _want_trace:
        _CACHE["last_result"] = res
    return out


# revision 25
# speedup vs baseline: 1.0949x; 1.0369x over previous
"""GQA decode attention (b=32, T=4096, 64 q-heads / 8 kv-heads) on 8 trn2 cores.

Tensor-parallel over heads: core i owns kv-head i (q-heads 8i..8i+7),
wqkv block i, KV-cache slice i, wo input-rows 1024i..1024(i+1); a
ReduceScatter finishes the row-parallel wo (host concatenates the shards).

Host-side layout prep (numerically equivalent, layout only):
  - RoPE is linear in q/k for a fixed position, so it is folded into the
    wqkv weight columns (q also absorbs the 1/sqrt(128) score scale).
  - K slice pre-transposed to [b, d, t] so score matmuls contract d on
    partitions; V / wqkv / wo packed partition-major for contiguous
    per-partition DMA runs.
  - Streamed operands cast to bf16 (fp32 PSUM accumulation throughout).
"""

import math
import sys

import numpy as np

sys.path.insert(0, "/opt/trn_rl_repo")

B = 32          # batch
D = 8192        # model dim
HD = 128        # head dim
H = 8           # q-heads per core
NKV = 8         # kv heads (= cores)
T = 4096        # kv length
NT = T // 128   # t-tiles
KD = D // 128   # k-tiles over model dim
BLK = 1280      # wqkv block per kv head (8*128 q | 128 k | 128 v)
KB = 8          # wqkv k-tiles batched per DMA

STREAM_BF16 = True   # stream K/V/weights as bf16 (fp32 accumulate)

_CACHE: dict = {}


def _build():
    from contextlib import ExitStack

    import concourse.tile as tile
    from concourse import bacc, mybir
    from concourse.masks import make_identity

    f32 = mybir.dt.float32
    dt = mybir.dt.bfloat16 if STREAM_BF16 else f32
    nc = bacc.Bacc("TRN2", target_bir_lowering=False, debug=False, num_devices=8)

    f16 = mybir.dt.float16
    xT = nc.dram_tensor("xT", [128, KD, B], dt, kind="ExternalInput")
    wq = nc.dram_tensor("wq", [128, KD, BLK], dt, kind="ExternalInput")
    kT = nc.dram_tensor("kT", [128, B, T], dt, kind="ExternalInput")
    vv = nc.dram_tensor("vv", [128, B, NT, HD], dt, kind="ExternalInput")
    woT = nc.dram_tensor("woT", [128, H, D], dt, kind="ExternalInput")
    out_ext = nc.dram_tensor("out", [B // 8, D], f16, kind="ExternalOutput")

    ExpF = mybir.ActivationFunctionType.Exp

    with tile.TileContext(nc) as tc, ExitStack() as ctx:
        cst = ctx.enter_context(tc.tile_pool(name="const", bufs=1))
        ident = cst.tile([128, 128], dt)
        make_identity(nc, ident[:])
        ident32 = cst.tile([8, 8], f32)
        make_identity(nc, ident32[:])
        ones = cst.tile([128, 1], dt)
        nc.vector.memset(ones[:], 1.0)

        # K/V stream pools sit below the phase-1 pools on the SBUF stack so
        # their first DMAs (no data deps) overlap the wqkv streaming.
        ktp = ctx.enter_context(tc.tile_pool(name="kt", bufs=4))
        vtp = ctx.enter_context(tc.tile_pool(name="vt", bufs=4))
        qT_sb = cst.tile([128, H, B], dt)       # q^T  [d, h, b]
        knT_sb = cst.tile([128, B], dt)         # k_new^T [d, b]
        vn_sb = cst.tile([B, HD], dt)           # v_new [b, d]
        attT_sb = cst.tile([128, H, B], dt)     # att^T [d, h, b]

        # ---------------- phase 1: fused qkv projection ----------------
        with (
            tc.tile_pool(name="w", bufs=3) as wpool,
            tc.tile_pool(name="xt", bufs=1) as xpool,
            tc.tile_pool(name="qps", bufs=1, space="PSUM") as qps,
            tc.tile_pool(name="m1", bufs=1) as m1,
            tc.tile_pool(name="tps", bufs=1, space="PSUM") as tps,
        ):
            xt = xpool.tile([128, KD, B], dt)
            nc.scalar.dma_start(xt[:], xT[:])
            ps_q1 = qps.tile([B, 512], f32)
            ps_q2 = qps.tile([B, 512], f32)
            ps_kv = qps.tile([B, 256], f32)
            for kk in range(0, KD, KB):
                wt = wpool.tile([128, KB, BLK], dt)
                nc.scalar.dma_start(wt[:], wq[:, kk:kk + KB, :])
                for k in range(KB):
                    lhs = xt[:, kk + k, :]
                    st, sp = kk + k == 0, kk + k == KD - 1
                    nc.tensor.matmul(ps_q1[:], lhs, wt[:, k, 0:512], start=st, stop=sp)
                    nc.tensor.matmul(ps_q2[:], lhs, wt[:, k, 512:1024], start=st, stop=sp)
                    nc.tensor.matmul(ps_kv[:], lhs, wt[:, k, 1024:1280], start=st, stop=sp)

            q_sb = m1.tile([B, 1024], dt)
            nc.vector.tensor_copy(q_sb[:, 0:512], ps_q1[:])
            nc.vector.tensor_copy(q_sb[:, 512:1024], ps_q2[:])
            kv_sb = m1.tile([B, 256], dt)
            nc.vector.tensor_copy(kv_sb[:], ps_kv[:])
            nc.vector.tensor_copy(vn_sb[:], kv_sb[:, 128:256])

            t_ps = tps.tile([128, H, B], dt)
            for h in range(H):
                nc.tensor.transpose(
                    t_ps[:, h, :], q_sb[:, h * 128:(h + 1) * 128], ident[0:B, 0:B]
                )
            nc.vector.tensor_copy(qT_sb[:], t_ps[:])
            t2_ps = tps.tile([128, B], dt)
            nc.tensor.transpose(t2_ps[:], kv_sb[:, 0:128], ident[0:B, 0:B])
            nc.vector.tensor_copy(knT_sb[:], t2_ps[:])

        wop = ctx.enter_context(tc.tile_pool(name="wo", bufs=16))

        # Warmup collective: R2 traces showed the first data-bearing cc op
        # runs ~3x slower than an identical immediate successor (32us vs
        # 11us for the same 256KB). Run a tiny ReduceScatter during the
        # attention streaming (cc engine is idle) so the real one at the
        # tail executes warm. Only the gpsimd engine waits on it.
        with (
            tc.tile_pool(name="warm", bufs=1) as wmp,
            tc.tile_pool(name="warmd", bufs=1, space="DRAM") as wmd,
        ):
            wm_sb = wmp.tile([B, 64], f16)
            nc.vector.memset(wm_sb[:], 0.0)
            wm_in = wmd.tile([B, 64], f16, name="wm_in")
            wm_out = wmd.tile([B // 8, 64], f16, name="wm_out")
            nc.sync.dma_start(wm_in[:], wm_sb[:])
            nc.gpsimd.collective_compute(
                "ReduceScatter",
                mybir.AluOpType.add,
                replica_groups=[list(range(8))],
                ins=[wm_in.opt()],
                outs=[wm_out.opt()],
            )

        # ---------------- phase 2: attention over batches ----------------
        with (
            tc.tile_pool(name="pr", bufs=3) as prp,
            tc.tile_pool(name="scps", bufs=3, space="PSUM") as scp,
            tc.tile_pool(name="ovps", bufs=2, space="PSUM") as ovp,
            tc.tile_pool(name="dnps", bufs=1, space="PSUM") as dnp,
            tc.tile_pool(name="atps", bufs=2, space="PSUM") as atp,
            tc.tile_pool(name="att", bufs=2) as attp,
        ):
            wt_tiles = []
            for b in range(B):
                # paced prefetch of wo weight tiles through the attention phase
                if b >= 2 and len(wt_tiles) < 16:
                  i = len(wt_tiles)
                  half, k = divmod(i, H)
                  wt = wop.tile([128, 4096], dt, name="wt", tag="wt")
                  eng = nc.sync if i % 2 == 0 else nc.scalar
                  eng.dma_start(wt[:], woT[:, k, half * 4096:(half + 1) * 4096])
                  wt_tiles.append(wt)
                kt_t = ktp.tile([128, T], dt, name="kt_t", tag="kt_t")
                nc.sync.dma_start(kt_t[:], kT[:, b, :])
                vt_t = vtp.tile([128, NT, HD], dt, name="vt_t", tag="vt_t")
                nc.scalar.dma_start(vt_t[:], vv[:, b, :, :])
                # overwrite column start_pos with the new (rope'd) k
                nc.vector.tensor_copy(kt_t[:, T - 1:T], knT_sb[:, b:b + 1])
                # overwrite row start_pos (= tile NT-1, partition 127) with new v
                nc.gpsimd.dma_start(vt_t[127:128, NT - 1, :], vn_sb[b:b + 1, :])

                sc = scp.tile([128, NT, H], f32)
                for j in range(NT):
                    nc.tensor.matmul(
                        sc[:, j, :], kt_t[:, j * 128:(j + 1) * 128], qT_sb[:, :, b],
                        start=True, stop=True,
                    )
                pr = prp.tile([128, NT, H], dt)
                nc.scalar.activation(pr[:], sc[:], ExpF)

                # denominator: collapse partitions with a ones-stationary matmul,
                # then fold the 32 tiles with a strided DVE reduce, then a tiny
                # PE transpose to land [8, 1] next to the PV output.
                dn1 = dnp.tile([1, NT * H], f32)
                nc.tensor.matmul(
                    dn1[:], ones[:], pr[:, :, :], start=True, stop=True
                )
                den8 = attp.tile([1, H], f32)
                nc.vector.reduce_sum(
                    den8[:], dn1.rearrange("p (t h) -> p h t", h=H),
                    axis=mybir.AxisListType.X,
                )

                ov = ovp.tile([H, HD + 1], f32)
                for j in range(NT):
                    nc.tensor.matmul(
                        ov[:, 0:HD], pr[:, j, :], vt_t[:, j, :],
                        start=(j == 0), stop=(j == NT - 1),
                    )
                nc.tensor.transpose(ov[:, HD:HD + 1], den8[:], ident32[0:1, 0:1])

                rec = attp.tile([H, 1], f32)
                nc.vector.reciprocal(rec[:], ov[:, HD:HD + 1])
                att_b = attp.tile([H, HD], dt)
                nc.vector.tensor_scalar_mul(att_b[:], ov[:, 0:HD], rec[:])
                at_ps = atp.tile([128, H], dt)
                nc.tensor.transpose(at_ps[:], att_b[:], ident[0:H, 0:H])
                nc.vector.tensor_copy(attT_sb[:, :, b], at_ps[:])

        # ---------------- phase 3: wo row-parallel + ReduceScatter ----------------
        # Column quarters pipeline through 2 PSUM slots (matmul q+1 while DVE
        # copies q). Partials cross the collective in fp16 (half the wire
        # bytes of fp32, 4x less rounding noise than bf16).
        with (
            tc.tile_pool(name="wops", bufs=2, space="PSUM") as wops,
            tc.tile_pool(name="ob", bufs=2) as obp,
            tc.tile_pool(name="dram", bufs=1, space="DRAM") as dram,
        ):
            cc_in = dram.tile([B, D], f16, name="cc_in")
            cc_out = dram.tile([B // 8, D], f16, name="cc_out")
            for quarter in range(4):
                half, hq = divmod(quarter, 2)
                psq = wops.tile([B, 2048], f32, name="wops", tag="wops")
                for k in range(H):
                    wt = wt_tiles[half * H + k]
                    for n in range(4):
                        cs = hq * 2048 + n * 512
                        nc.tensor.matmul(
                            psq[:, n * 512:(n + 1) * 512], attT_sb[:, k, :],
                            wt[:, cs:cs + 512],
                            start=(k == 0), stop=(k == H - 1),
                        )
                ob = obp.tile([B, 2048], f16, name="ob", tag="ob")
                nc.vector.tensor_copy(ob[:], psq[:])
                nc.sync.dma_start(
                    cc_in[:, quarter * 2048:(quarter + 1) * 2048], ob[:]
                )
            nc.gpsimd.collective_compute(
                "ReduceScatter",
                mybir.AluOpType.add,
                replica_groups=[list(range(8))],
                ins=[cc_in.opt()],
                outs=[cc_out.opt()],
            )
            nc.sync.dma_start(out_ext[:], cc_out[:])

    nc.compile()
    return nc


def _prep_inputs(x, cache_k, cache_v, wqkv_w, wo_w, freqs_cos, freqs_sin):
    if STREAM_BF16:
        import ml_dtypes

        sdt = ml_dtypes.bfloat16
    else:
        sdt = np.float32
    cos = np.asarray(freqs_cos, np.float32).reshape(-1)[:64]
    sin = np.asarray(freqs_sin, np.float32).reshape(-1)[:64]
    x = np.asarray(x, np.float32).reshape(B, D)
    # x^T packed tile-major: xT[p, k, b] = x[b, 128k+p]
    xT = np.ascontiguousarray(x.reshape(B, KD, 128).transpose(2, 1, 0)).astype(sdt)

    wqkv_w = np.asarray(wqkv_w, np.float32)
    scale = 1.0 / math.sqrt(HD)
    in_maps = []
    for c in range(8):
        W = wqkv_w[:, c * BLK:(c + 1) * BLK].copy()
        q = W[:, :1024].reshape(D, H, 64, 2)
        q0 = q[..., 0].copy()
        q1 = q[..., 1].copy()
        q[..., 0] = (q0 * cos - q1 * sin) * scale
        q[..., 1] = (q0 * sin + q1 * cos) * scale
        k = W[:, 1024:1152].reshape(D, 64, 2)
        k0 = k[..., 0].copy()
        k1 = k[..., 1].copy()
        k[..., 0] = k0 * cos - k1 * sin
        k[..., 1] = k0 * sin + k1 * cos
        # partition-major: wq_pm[p, kt, :] = W[kt*128+p, :]
        W_pm = np.ascontiguousarray(
            W.reshape(KD, 128, BLK).transpose(1, 0, 2)
        ).astype(sdt)

        kTc = np.ascontiguousarray(
            np.asarray(cache_k[:, :, c, :], np.float32).transpose(2, 0, 1)
        ).astype(sdt)  # [128, B, T]
        vc = np.ascontiguousarray(
            np.asarray(cache_v[:, :, c, :], np.float32)
            .reshape(B, NT, 128, HD)
            .transpose(2, 0, 1, 3)
        ).astype(sdt)  # [128, B, NT, HD]
        woTc = np.asarray(wo_w[:, c * 1024:(c + 1) * 1024], np.float32).T  # [1024, D]
        woT_pm = np.ascontiguousarray(
            woTc.reshape(H, 128, D).transpose(1, 0, 2)
        ).astype(sdt)  # [128, H, D]
        in_maps.append({
            "xT": xT, "wq": W_pm, "kT": kTc, "vv": vc, "woT": woT_pm,
        })
    return in_maps


def kernel(x, cache_k, cache_v, wqkv_w, wo_w, freqs_cos, freqs_sin, mask,
           start_pos, _want_trace=False, **_unused):
    from concourse.bass_utils import run_bass_kernel_spmd

    sp = int(np.asarray(start_pos))
    assert sp == T - 1, f"kernel compiled for start_pos={T - 1}, got {sp}"

    if "nc" not in _CACHE:
        _CACHE["nc"] = _build()
    nc = _CACHE["nc"]

    in_maps = _prep_inputs(x, cache_k, cache_v, wqkv_w, wo_w, freqs_cos, freqs_sin)
    res = run_bass_kernel_spmd(nc, in_maps, list(range(8)), trace=_want_trace)
    # ReduceScatter leaves rank i holding reduced rows 4i..4(i+1): concatenate
    out = np.concatenate(
        [np.asarray(res.results[i]["out"], np.float32) for i in range(8)], axis=0
    )
    out = out.reshape(B, 1, D)
    if _want_trace:
        _CACHE["last_result"] = res
    return out

